# revision 1
# baseline (speedup 1.0000x reference)
"""Cubemap bilinear sampling (nn_CubemapEncoder) — TRN2 Bass kernel.

Contract: kernel(inputs, tex, failv) -> [B, 6] f32, with
  inputs [4194304, 3] f32, tex [6, 6, 256, 256] f32, failv [6] f32.

Strategy (8 NeuronCores, data-parallel over rays; texture replicated):
  - Host precomputes a quad-diff table [6*256*256+1, 24] f32: per
    (face, v0, u0) site A=t00, B=t01-t00, C=t10-t00, D=t00+t11-t01-t10
    (6 channels each, clamp-to-edge folded in; last row = failv).
  - Fused custom DVE ops compute per ray: face/site index, wu, wv, wu*wv.
  - One 96B indirect-DMA gather per ray fetches its quad-diff row.
  - Blend: out = A + wu*B + wv*C + (wu*wv)*D.
"""

import os
import sys

import numpy as np

for _p in ("/opt/trn_rl_repo", "/root/.axon_site/_ro/trn_rl_repo"):
    if os.path.isdir(_p) and _p not in sys.path:
        sys.path.insert(0, _p)

import concourse.bass as bass
import concourse.bacc as bacc
import concourse.mybir as mybir
import concourse.tile as tile
import concourse.dve_ops as dve_ops_mod
from concourse.dve_ops import DveOp
from concourse.dve_spec import (
    Spec,
    Src0,
    Src1,
    C0,
    C1,
    C2,
    Zero,
    maxx,
    minn,
    select,
    lower,
    _has_src1,
)
from concourse.dve_uop import DveOpSpec
from concourse import bass_utils

F32 = mybir.dt.float32
I32 = mybir.dt.int32
U8 = mybir.dt.uint8

B_TOTAL = 4194304
N_CORES = 8
PER_CORE = B_TOTAL // N_CORES
T_FREE = 256

L = 256
C = 6
SITES = 6 * L * L
FAIL_SITE = float(SITES)
TABLE_ROWS = SITES + 1
ROW = 24
EPS = 1e-12
MAGIC = 8388608.0  # 2^23: RNE-magic floor for values in [0, 2^23)

SB_Z = (5 * 65536 - 256.0, 4 * 65536 - 256.0)
SB_Y = (3 * 65536 - 256.0, 2 * 65536 - 256.0)
SB_X = (1 * 65536 - 256.0, 0 * 65536 - 256.0)


# ---- custom fused DVE ops -------------------------------------------------


def _register(name, spec):
    existing = {op.name: op for op in dve_ops_mod.OPS}
    if name in existing:
        return existing[name]
    row = max(dve_ops_mod._SUB_OPCODE_FOR_NAME.values()) + 1
    assert row < 0x20, f"custom DVE row overflow: {row}"
    dve_ops_mod._SUB_OPCODE_FOR_NAME[name] = row
    shas = {}
    for ver in ("v3", "v4"):
        uops = lower(spec, ver=ver)
        s = DveOpSpec(name=name, opcode=row, uops=uops, rd1_en=_has_src1(spec))
        shas[ver] = s.sha(ver)
    op = DveOp(name, spec, subdim=False, uops_sha=shas)
    dve_ops_mod.OPS.append(op)
    dve_ops_mod.CUSTOM_DVE_SPECS[name] = spec
    return op


def _np_floor_magic(t):
    t = np.asarray(t, np.float32)
    r = ((t + np.float32(MAGIC)).astype(np.float32) - np.float32(MAGIC)).astype(
        np.float32
    )
    return (r - (r > t).astype(np.float32)).astype(np.float32)


def _frac_ref(in0, in1, c0, c1, c2):
    t = (np.asarray(in0, np.float32) + np.float32(c0)).astype(np.float32)
    return (t - _np_floor_magic(t)).astype(np.float32)


def _floorv_ref(in0, in1, c0, c1, c2):
    t = (np.asarray(in0, np.float32) + np.float32(c0)).astype(np.float32)
    return (_np_floor_magic(t) - np.float32(c0)).astype(np.float32)


def _siteu_ref(in0, in1, c0, c1, c2):
    t = (np.asarray(in0, np.float32) + np.float32(c0)).astype(np.float32)
    f = _np_floor_magic(t)
    h = np.minimum(np.maximum(f, np.float32(c0)), np.float32(c2))
    return (h + in1).astype(np.float32)


_t = Src0 + C0
_a = _t + C1
_r = _a - C1
_f = _r - (_r > _t)

OPS_DEFS = {}


def _init_ops():
    if OPS_DEFS:
        return
    OPS_DEFS["ABS2MAX"] = _register(
        "CM_ABS2MAX",
        Spec(
            body=maxx(maxx(Src0, Zero - Src0), maxx(Src1, Zero - Src1)),
            reference=lambda i0, i1, c0, c1, c2: np.maximum(
                np.abs(i0), np.abs(i1)
            ).astype(np.float32),
        ),
    )
    OPS_DEFS["ZMAXEPS"] = _register(
        "CM_ZMAXEPS",
        Spec(
            body=maxx(maxx(maxx(Src0, Zero - Src0), Src1), C0),
            reference=lambda i0, i1, c0, c1, c2: np.maximum(
                np.maximum(np.abs(i0), i1), np.float32(c0)
            ).astype(np.float32),
        ),
    )
    OPS_DEFS["ABSGE"] = _register(
        "CM_ABSGE",
        Spec(
            body=maxx(Src0, Zero - Src0) >= Src1,
            reference=lambda i0, i1, c0, c1, c2: (np.abs(i0) >= i1).astype(
                np.float32
            ),
        ),
    )
    OPS_DEFS["SELSIGN"] = _register(
        "CM_SELSIGN",
        Spec(
            body=select(Src0 >= Zero, Src1, Zero - Src1),
            reference=lambda i0, i1, c0, c1, c2: np.where(i0 >= 0, i1, -i1).astype(
                np.float32
            ),
        ),
    )
    OPS_DEFS["SELNEGSIGN"] = _register(
        "CM_SELNEGSIGN",
        Spec(
            body=select(Src0 >= Zero, Zero - Src1, Src1),
            reference=lambda i0, i1, c0, c1, c2: np.where(i0 >= 0, -i1, i1).astype(
                np.float32
            ),
        ),
    )
    OPS_DEFS["SITESEL"] = _register(
        "CM_SITESEL",
        Spec(
            body=select(Src0 < Zero, C0, C1),
            reference=lambda i0, i1, c0, c1, c2: np.where(
                i0 < 0, np.float32(c0), np.float32(c1)
            ).astype(np.float32),
        ),
    )
    OPS_DEFS["MULSCALEBIAS"] = _register(
        "CM_MULSCALEBIAS",
        Spec(
            body=(Src0 * Src1) * C0 + C1,
            reference=lambda i0, i1, c0, c1, c2: (
                (i0 * i1) * np.float32(c0) + np.float32(c1)
            ).astype(np.float32),
        ),
    )
    OPS_DEFS["FRAC"] = _register("CM_FRAC", Spec(body=_t - _f, reference=_frac_ref))
    OPS_DEFS["FLOORV"] = _register(
        "CM_FLOORV", Spec(body=_f - C0, reference=_floorv_ref)
    )
    OPS_DEFS["SITEV"] = _register(
        "CM_SITEV",
        Spec(
            body=minn(maxx(Src0, Zero), C0) * C1 + Src1,
            reference=lambda i0, i1, c0, c1, c2: (
                np.minimum(np.maximum(i0, 0), np.float32(c0)) * np.float32(c1) + i1
            ).astype(np.float32),
        ),
    )
    OPS_DEFS["SITEU8"] = _register(
        "CM_SITEU8", Spec(body=minn(maxx(_f, C0), C2) + Src1, reference=_siteu_ref)
    )
    OPS_DEFS["SITEC"] = _register(
        "CM_SITEC",
        Spec(
            body=select(Src1 > C0, Src0, C1),
            reference=lambda i0, i1, c0, c1, c2: np.where(
                i1 > np.float32(c0), i0, np.float32(c1)
            ).astype(np.float32),
        ),
    )


# ---- host table prep ------------------------------------------------------


def build_table(tex, failv):
    texT = np.transpose(np.asarray(tex, np.float64), (0, 2, 3, 1))
    nxt = np.minimum(np.arange(L) + 1, L - 1)
    t00 = texT
    t01 = texT[:, :, nxt, :]
    t10 = texT[:, nxt, :, :]
    t11 = texT[:, nxt, :, :][:, :, nxt, :]
    quad = np.concatenate(
        [t00, t01 - t00, t10 - t00, (t00 + t11) - (t01 + t10)], axis=3
    )
    table = quad.reshape(-1, ROW)
    fail = np.zeros((1, ROW))
    fail[0, :C] = np.asarray(failv, np.float64)
    return np.ascontiguousarray(
        np.concatenate([table, fail], axis=0).astype(np.float32)
    )


# ---- device program -------------------------------------------------------


def build_nc(rays_per_core, t_free, bufs=2):
    _init_ops()
    P = 128
    per_tile = P * t_free
    assert rays_per_core % per_tile == 0
    n_tiles = rays_per_core // per_tile

    nc = bacc.Bacc("TRN2", target_bir_lowering=False, debug=False)
    rays_d = nc.dram_tensor("rays", [rays_per_core, 3], F32, kind="ExternalInput").ap()
    table_d = nc.dram_tensor(
        "table", [TABLE_ROWS, ROW], F32, kind="ExternalInput"
    ).ap()
    out_d = nc.dram_tensor("out", [rays_per_core, 6], F32, kind="ExternalOutput").ap()

    rays_v = rays_d.rearrange("(n p t) c -> n p (t c)", p=P, t=t_free)
    out_v = out_d.rearrange("(n p t) c -> n p (t c)", p=P, t=t_free)
    O = OPS_DEFS

    with tile.TileContext(nc) as tc:
        with tc.tile_pool(name="work", bufs=bufs) as pool, tc.tile_pool(
            name="big", bufs=2
        ) as bigpool:
            v = nc.vector
            s = nc.scalar
            g_ = nc.gpsimd
            t = t_free
            MUL, ADD = mybir.AluOpType.mult, mybir.AluOpType.add

            for i in range(n_tiles):
                rt = pool.tile([P, 3 * t], F32, tag="rt")
                nc.sync.dma_start(out=rt[:], in_=rays_v[i])
                r3 = rt[:].rearrange("p (t c) -> p t c", c=3)
                x, y, z = r3[:, :, 0], r3[:, :, 1], r3[:, :, 2]

                mxy = pool.tile([P, t], F32, tag="mxy")
                mac = pool.tile([P, t], F32, tag="mac")
                is_x = pool.tile([P, t], U8, tag="is_x")
                e_y = pool.tile([P, t], U8, tag="e_y")
                v._custom_dve(O["ABS2MAX"], out=mxy[:], in0=x, in1=y)
                v._custom_dve(O["ZMAXEPS"], out=mac[:], in0=z, in1=mxy[:], s0=0.0)
                v._custom_dve(O["ABSGE"], out=is_x[:], in0=x, in1=mac[:])
                v._custom_dve(O["ABSGE"], out=e_y[:], in0=y, in1=mac[:])

                r0 = pool.tile([P, t], F32, tag="r0")
                macc = pool.tile([P, t], F32, tag="macc")
                inv = pool.tile([P, t], F32, tag="inv")
                v.tensor_scalar_max(macc[:], mac[:], EPS)
                v.reciprocal_approx_accurate(out=inv[:], in_=macc[:], scratch=r0[:])

                sc = pool.tile([P, t], F32, tag="sc")
                t5 = pool.tile([P, t], F32, tag="t5")
                v._custom_dve(O["SELSIGN"], out=sc[:], in0=z, in1=x)
                v.copy_predicated(sc[:], e_y[:], x)
                v._custom_dve(O["SELNEGSIGN"], out=t5[:], in0=x, in1=z)
                v.copy_predicated(sc[:], is_x[:], t5[:])

                negy = pool.tile([P, t], F32, tag="negy")
                tcs = pool.tile([P, t], F32, tag="tcs")
                t6 = pool.tile([P, t], F32, tag="t6")
                s.mul(negy[:], y, -1.0)
                s.mul(tcs[:], y, -1.0)
                v._custom_dve(O["SELSIGN"], out=t6[:], in0=y, in1=z)
                v.copy_predicated(tcs[:], e_y[:], t6[:])
                v.copy_predicated(tcs[:], is_x[:], negy[:])

                sb = pool.tile([P, t], F32, tag="sb")
                t7 = pool.tile([P, t], F32, tag="t7")
                v._custom_dve(O["SITESEL"], out=sb[:], in0=z, s0=SB_Z[0], s1=SB_Z[1])
                v._custom_dve(O["SITESEL"], out=t7[:], in0=y, s0=SB_Y[0], s1=SB_Y[1])
                v.copy_predicated(sb[:], e_y[:], t7[:])
                v._custom_dve(O["SITESEL"], out=t7[:], in0=x, s0=SB_X[0], s1=SB_X[1])
                v.copy_predicated(sb[:], is_x[:], t7[:])

                pu = pool.tile([P, t], F32, tag="pu")
                pv = pool.tile([P, t], F32, tag="pv")
                v._custom_dve(
                    O["MULSCALEBIAS"], out=pu[:], in0=sc[:], in1=inv[:],
                    s0=128.0, s1=127.5,
                )
                v._custom_dve(
                    O["MULSCALEBIAS"], out=pv[:], in0=tcs[:], in1=inv[:],
                    s0=128.0, s1=127.5,
                )

                wu = pool.tile([P, t], F32, tag="wu")
                wv = pool.tile([P, t], F32, tag="wv")
                wuv = pool.tile([P, t], F32, tag="wuv")
                v0f = pool.tile([P, t], F32, tag="v0f")
                v._custom_dve(O["FRAC"], out=wu[:], in0=pu[:], s0=256.0, s1=MAGIC)
                v._custom_dve(O["FRAC"], out=wv[:], in0=pv[:], s0=256.0, s1=MAGIC)
                v.tensor_tensor(out=wuv[:], in0=wu[:], in1=wv[:], op=MUL)
                v._custom_dve(O["FLOORV"], out=v0f[:], in0=pv[:], s0=256.0, s1=MAGIC)

                sv = pool.tile([P, t], F32, tag="sv")
                suf = pool.tile([P, t], F32, tag="suf")
                site_i = pool.tile([P, t], I32, tag="site_i")
                v._custom_dve(
                    O["SITEV"], out=sv[:], in0=v0f[:], in1=sb[:], s0=255.0, s1=256.0
                )
                v._custom_dve(
                    O["SITEU8"], out=suf[:], in0=pu[:], in1=sv[:],
                    s0=256.0, s1=MAGIC, imm2=511.0,
                )
                v._custom_dve(
                    O["SITEC"], out=suf[:], in0=suf[:], in1=mac[:],
                    s0=0.0, s1=FAIL_SITE,
                )
                v.tensor_copy(out=site_i[:], in_=suf[:])

                gq = bigpool.tile([P, t, ROW], F32, tag="gq")
                for j in range(t):
                    g_.indirect_dma_start(
                        out=gq[:, j, :],
                        out_offset=None,
                        in_=table_d[:],
                        in_offset=bass.IndirectOffsetOnAxis(
                            ap=site_i[:, j : j + 1], axis=0
                        ),
                    )

                A = gq[:, :, 0:6]
                Bd = gq[:, :, 6:12]
                Cd = gq[:, :, 12:18]
                Dd = gq[:, :, 18:24]
                wub = wu[:, :, None].to_broadcast([P, t, 6])
                wvb = wv[:, :, None].to_broadcast([P, t, 6])
                wuvb = wuv[:, :, None].to_broadcast([P, t, 6])

                m1 = pool.tile([P, t, 6], F32, tag="m1")
                m2 = pool.tile([P, t, 6], F32, tag="m2")
                m3 = pool.tile([P, t, 6], F32, tag="m3")
                outt = pool.tile([P, t, 6], F32, tag="outt")
                v.tensor_tensor(out=m1[:], in0=Bd, in1=wub, op=MUL)
                v.tensor_tensor(out=m2[:], in0=Cd, in1=wvb, op=MUL)
                v.tensor_tensor(out=m3[:], in0=Dd, in1=wuvb, op=MUL)
                v.tensor_tensor(out=m1[:], in0=m1[:], in1=A, op=ADD)
                v.tensor_tensor(out=m2[:], in0=m2[:], in1=m3[:], op=ADD)
                v.tensor_tensor(out=outt[:], in0=m1[:], in1=m2[:], op=ADD)

                nc.sync.dma_start(
                    out=out_v[i], in_=outt[:].rearrange("p t c -> p (t c)")
                )

    nc.compile()
    return nc


_NC_CACHE = {}


def _get_nc():
    key = (PER_CORE, T_FREE)
    if key not in _NC_CACHE:
        _NC_CACHE[key] = build_nc(PER_CORE, T_FREE, bufs=3)
    return _NC_CACHE[key]


def kernel(inputs, tex, failv, _trace=False, _trace_kwargs=None):
    rays = np.ascontiguousarray(np.asarray(inputs, np.float32))
    assert rays.shape == (B_TOTAL, 3), rays.shape
    table = build_table(np.asarray(tex, np.float32), np.asarray(failv, np.float32))
    nc = _get_nc()
    in_maps = [
        {"rays": rays[i * PER_CORE : (i + 1) * PER_CORE], "table": table}
        for i in range(N_CORES)
    ]
    try:
        res = bass_utils.run_bass_kernel_spmd(
            nc, in_maps, list(range(N_CORES)), trace=_trace,
            **(_trace_kwargs or {}),
        )
    except Exception:
        if not _trace:
            raise
        res = bass_utils.run_bass_kernel_spmd(nc, in_maps, list(range(N_CORES)))
    out = np.concatenate([r["out"] for r in res.results], axis=0)
    if _trace:
        kernel.last_result = res
    return out



# revision 4
# speedup vs baseline: 1.0077x; 1.0077x over previous
"""Cubemap bilinear sampling (nn_CubemapEncoder) — TRN2 Bass kernel.

Contract: kernel(inputs, tex, failv) -> [B, 6] f32, with
  inputs [4194304, 3] f32, tex [6, 6, 256, 256] f32, failv [6] f32.

Strategy (8 NeuronCores, data-parallel over rays; texture replicated):
  - Host precomputes a quad-diff table [6*256*256+1, 24] f32: per
    (face, v0, u0) site A=t00, B=t01-t00, C=t10-t00, D=t00+t11-t01-t10
    (6 channels each, clamp-to-edge folded in; last row = failv).
  - Fused custom DVE ops compute per ray: face/site index, wu, wv, wu*wv.
  - One 96B indirect-DMA gather per ray fetches its quad-diff row.
  - Blend: out = A + wu*B + wv*C + (wu*wv)*D.
"""

import os
import sys

import numpy as np

for _p in ("/opt/trn_rl_repo", "/root/.axon_site/_ro/trn_rl_repo"):
    if os.path.isdir(_p) and _p not in sys.path:
        sys.path.insert(0, _p)

import concourse.bass as bass
import concourse.bacc as bacc
import concourse.mybir as mybir
import concourse.tile as tile
import concourse.dve_ops as dve_ops_mod
from concourse.dve_ops import DveOp
from concourse.dve_spec import (
    Spec,
    Src0,
    Src1,
    C0,
    C1,
    C2,
    Zero,
    maxx,
    minn,
    select,
    lower,
    _has_src1,
)
from concourse.dve_uop import DveOpSpec
from concourse import bass_utils

F32 = mybir.dt.float32
F16 = mybir.dt.float16
I32 = mybir.dt.int32
U8 = mybir.dt.uint8

B_TOTAL = 4194304
N_CORES = 8
PER_CORE = B_TOTAL // N_CORES
T_FREE = 256

L = 256
C = 6
SITES = 6 * L * L
FAIL_SITE = float(SITES)
TABLE_ROWS = SITES + 1
ROW = 24
EPS = 1e-12
MAGIC = 8388608.0  # 2^23: RNE-magic floor for values in [0, 2^23)

SB_Z = (5 * 65536 - 256.0, 4 * 65536 - 256.0)
SB_Y = (3 * 65536 - 256.0, 2 * 65536 - 256.0)
SB_X = (1 * 65536 - 256.0, 0 * 65536 - 256.0)


# ---- custom fused DVE ops -------------------------------------------------


def _register(name, spec):
    existing = {op.name: op for op in dve_ops_mod.OPS}
    if name in existing:
        return existing[name]
    row = max(dve_ops_mod._SUB_OPCODE_FOR_NAME.values()) + 1
    assert row < 0x20, f"custom DVE row overflow: {row}"
    dve_ops_mod._SUB_OPCODE_FOR_NAME[name] = row
    shas = {}
    for ver in ("v3", "v4"):
        uops = lower(spec, ver=ver)
        s = DveOpSpec(name=name, opcode=row, uops=uops, rd1_en=_has_src1(spec))
        shas[ver] = s.sha(ver)
    op = DveOp(name, spec, subdim=False, uops_sha=shas)
    dve_ops_mod.OPS.append(op)
    dve_ops_mod.CUSTOM_DVE_SPECS[name] = spec
    return op


def _np_floor_magic(t):
    t = np.asarray(t, np.float32)
    r = ((t + np.float32(MAGIC)).astype(np.float32) - np.float32(MAGIC)).astype(
        np.float32
    )
    return (r - (r > t).astype(np.float32)).astype(np.float32)


def _frac_ref(in0, in1, c0, c1, c2):
    t = (np.asarray(in0, np.float32) + np.float32(c0)).astype(np.float32)
    return (t - _np_floor_magic(t)).astype(np.float32)


def _floorv_ref(in0, in1, c0, c1, c2):
    t = (np.asarray(in0, np.float32) + np.float32(c0)).astype(np.float32)
    return (_np_floor_magic(t) - np.float32(c0)).astype(np.float32)


def _siteu_ref(in0, in1, c0, c1, c2):
    t = (np.asarray(in0, np.float32) + np.float32(c0)).astype(np.float32)
    f = _np_floor_magic(t)
    h = np.minimum(np.maximum(f, np.float32(c0)), np.float32(c2))
    return (h + in1).astype(np.float32)


_t = Src0 + C0
_a = _t + C1
_r = _a - C1
_f = _r - (_r > _t)

OPS_DEFS = {}


def _init_ops():
    if OPS_DEFS:
        return
    OPS_DEFS["ABS2MAX"] = _register(
        "CM_ABS2MAX",
        Spec(
            body=maxx(maxx(Src0, Zero - Src0), maxx(Src1, Zero - Src1)),
            reference=lambda i0, i1, c0, c1, c2: np.maximum(
                np.abs(i0), np.abs(i1)
            ).astype(np.float32),
        ),
    )
    OPS_DEFS["ZMAXEPS"] = _register(
        "CM_ZMAXEPS",
        Spec(
            body=maxx(maxx(maxx(Src0, Zero - Src0), Src1), C0),
            reference=lambda i0, i1, c0, c1, c2: np.maximum(
                np.maximum(np.abs(i0), i1), np.float32(c0)
            ).astype(np.float32),
        ),
    )
    OPS_DEFS["ABSGE"] = _register(
        "CM_ABSGE",
        Spec(
            body=maxx(Src0, Zero - Src0) >= Src1,
            reference=lambda i0, i1, c0, c1, c2: (np.abs(i0) >= i1).astype(
                np.float32
            ),
        ),
    )
    OPS_DEFS["SELSIGN"] = _register(
        "CM_SELSIGN",
        Spec(
            body=select(Src0 >= Zero, Src1, Zero - Src1),
            reference=lambda i0, i1, c0, c1, c2: np.where(i0 >= 0, i1, -i1).astype(
                np.float32
            ),
        ),
    )
    OPS_DEFS["SELNEGSIGN"] = _register(
        "CM_SELNEGSIGN",
        Spec(
            body=select(Src0 >= Zero, Zero - Src1, Src1),
            reference=lambda i0, i1, c0, c1, c2: np.where(i0 >= 0, -i1, i1).astype(
                np.float32
            ),
        ),
    )
    OPS_DEFS["SITESEL"] = _register(
        "CM_SITESEL",
        Spec(
            body=select(Src0 < Zero, C0, C1),
            reference=lambda i0, i1, c0, c1, c2: np.where(
                i0 < 0, np.float32(c0), np.float32(c1)
            ).astype(np.float32),
        ),
    )
    OPS_DEFS["MULSCALEBIAS"] = _register(
        "CM_MULSCALEBIAS",
        Spec(
            body=(Src0 * Src1) * C0 + C1,
            reference=lambda i0, i1, c0, c1, c2: (
                (i0 * i1) * np.float32(c0) + np.float32(c1)
            ).astype(np.float32),
        ),
    )
    OPS_DEFS["FRAC"] = _register("CM_FRAC", Spec(body=_t - _f, reference=_frac_ref))
    OPS_DEFS["FLOORV"] = _register(
        "CM_FLOORV", Spec(body=_f - C0, reference=_floorv_ref)
    )
    OPS_DEFS["SITEV"] = _register(
        "CM_SITEV",
        Spec(
            body=minn(maxx(Src0, Zero), C0) * C1 + Src1,
            reference=lambda i0, i1, c0, c1, c2: (
                np.minimum(np.maximum(i0, 0), np.float32(c0)) * np.float32(c1) + i1
            ).astype(np.float32),
        ),
    )
    OPS_DEFS["SITEU8"] = _register(
        "CM_SITEU8", Spec(body=minn(maxx(_f, C0), C2) + Src1, reference=_siteu_ref)
    )
    OPS_DEFS["SITEC"] = _register(
        "CM_SITEC",
        Spec(
            body=select(Src1 > C0, Src0, C1),
            reference=lambda i0, i1, c0, c1, c2: np.where(
                i1 > np.float32(c0), i0, np.float32(c1)
            ).astype(np.float32),
        ),
    )


# ---- host table prep ------------------------------------------------------


def build_table(tex, failv):
    texT = np.transpose(np.asarray(tex, np.float64), (0, 2, 3, 1))
    nxt = np.minimum(np.arange(L) + 1, L - 1)
    t00 = texT
    t01 = texT[:, :, nxt, :]
    t10 = texT[:, nxt, :, :]
    t11 = texT[:, nxt, :, :][:, :, nxt, :]
    quad = np.concatenate(
        [t00, t01 - t00, t10 - t00, (t00 + t11) - (t01 + t10)], axis=3
    )
    table = quad.reshape(-1, ROW)
    fail = np.zeros((1, ROW))
    fail[0, :C] = np.asarray(failv, np.float64)
    return np.ascontiguousarray(
        np.concatenate([table, fail], axis=0).astype(np.float16)
    )


# ---- device program -------------------------------------------------------


def build_nc(rays_per_core, t_free, bufs=2):
    _init_ops()
    P = 128
    per_tile = P * t_free
    assert rays_per_core % per_tile == 0
    n_tiles = rays_per_core // per_tile

    nc = bacc.Bacc("TRN2", target_bir_lowering=False, debug=False)
    rays_d = nc.dram_tensor("rays", [rays_per_core, 3], F32, kind="ExternalInput").ap()
    table_d = nc.dram_tensor(
        "table", [TABLE_ROWS, ROW], F16, kind="ExternalInput"
    ).ap()
    out_d = nc.dram_tensor("out", [rays_per_core, 6], F32, kind="ExternalOutput").ap()

    rays_v = rays_d.rearrange("(n p t) c -> n p (t c)", p=P, t=t_free)
    out_v = out_d.rearrange("(n p t) c -> n p (t c)", p=P, t=t_free)
    O = OPS_DEFS

    with tile.TileContext(nc) as tc:
        with tc.tile_pool(name="work", bufs=bufs) as pool, tc.tile_pool(
            name="big", bufs=3
        ) as bigpool:
            v = nc.vector
            s = nc.scalar
            g_ = nc.gpsimd
            t = t_free
            MUL, ADD = mybir.AluOpType.mult, mybir.AluOpType.add

            for i in range(n_tiles):
                rt = pool.tile([P, 3 * t], F32, tag="rt")
                nc.sync.dma_start(out=rt[:], in_=rays_v[i])
                r3 = rt[:].rearrange("p (t c) -> p t c", c=3)
                x, y, z = r3[:, :, 0], r3[:, :, 1], r3[:, :, 2]

                mxy = pool.tile([P, t], F32, tag="mxy")
                mac = pool.tile([P, t], F32, tag="mac")
                is_x = pool.tile([P, t], U8, tag="is_x")
                e_y = pool.tile([P, t], U8, tag="e_y")
                v._custom_dve(O["ABS2MAX"], out=mxy[:], in0=x, in1=y)
                v._custom_dve(O["ZMAXEPS"], out=mac[:], in0=z, in1=mxy[:], s0=0.0)
                v._custom_dve(O["ABSGE"], out=is_x[:], in0=x, in1=mac[:])
                v._custom_dve(O["ABSGE"], out=e_y[:], in0=y, in1=mac[:])

                r0 = pool.tile([P, t], F32, tag="r0")
                macc = pool.tile([P, t], F32, tag="macc")
                inv = pool.tile([P, t], F32, tag="inv")
                v.tensor_scalar_max(macc[:], mac[:], EPS)
                v.reciprocal_approx_accurate(out=inv[:], in_=macc[:], scratch=r0[:])

                sc = pool.tile([P, t], F32, tag="sc")
                t5 = pool.tile([P, t], F32, tag="t5")
                v._custom_dve(O["SELSIGN"], out=sc[:], in0=z, in1=x)
                v.copy_predicated(sc[:], e_y[:], x)
                v._custom_dve(O["SELNEGSIGN"], out=t5[:], in0=x, in1=z)
                v.copy_predicated(sc[:], is_x[:], t5[:])

                negy = pool.tile([P, t], F32, tag="negy")
                tcs = pool.tile([P, t], F32, tag="tcs")
                t6 = pool.tile([P, t], F32, tag="t6")
                s.mul(negy[:], y, -1.0)
                s.mul(tcs[:], y, -1.0)
                v._custom_dve(O["SELSIGN"], out=t6[:], in0=y, in1=z)
                v.copy_predicated(tcs[:], e_y[:], t6[:])
                v.copy_predicated(tcs[:], is_x[:], negy[:])

                sb = pool.tile([P, t], F32, tag="sb")
                t7 = pool.tile([P, t], F32, tag="t7")
                v._custom_dve(O["SITESEL"], out=sb[:], in0=z, s0=SB_Z[0], s1=SB_Z[1])
                v._custom_dve(O["SITESEL"], out=t7[:], in0=y, s0=SB_Y[0], s1=SB_Y[1])
                v.copy_predicated(sb[:], e_y[:], t7[:])
                v._custom_dve(O["SITESEL"], out=t7[:], in0=x, s0=SB_X[0], s1=SB_X[1])
                v.copy_predicated(sb[:], is_x[:], t7[:])

                pu = pool.tile([P, t], F32, tag="pu")
                pv = pool.tile([P, t], F32, tag="pv")
                v._custom_dve(
                    O["MULSCALEBIAS"], out=pu[:], in0=sc[:], in1=inv[:],
                    s0=128.0, s1=127.5,
                )
                v._custom_dve(
                    O["MULSCALEBIAS"], out=pv[:], in0=tcs[:], in1=inv[:],
                    s0=128.0, s1=127.5,
                )

                wu = pool.tile([P, t], F32, tag="wu")
                wv = pool.tile([P, t], F32, tag="wv")
                v0f = pool.tile([P, t], F32, tag="v0f")
                v._custom_dve(O["FRAC"], out=wu[:], in0=pu[:], s0=256.0, s1=MAGIC)
                v._custom_dve(O["FRAC"], out=wv[:], in0=pv[:], s0=256.0, s1=MAGIC)
                v._custom_dve(O["FLOORV"], out=v0f[:], in0=pv[:], s0=256.0, s1=MAGIC)

                sv = pool.tile([P, t], F32, tag="sv")
                suf = pool.tile([P, t], F32, tag="suf")
                site_i = pool.tile([P, t], I32, tag="site_i")
                v._custom_dve(
                    O["SITEV"], out=sv[:], in0=v0f[:], in1=sb[:], s0=255.0, s1=256.0
                )
                v._custom_dve(
                    O["SITEU8"], out=suf[:], in0=pu[:], in1=sv[:],
                    s0=256.0, s1=MAGIC, imm2=511.0,
                )
                v._custom_dve(
                    O["SITEC"], out=suf[:], in0=suf[:], in1=mac[:],
                    s0=0.0, s1=FAIL_SITE,
                )
                v.tensor_copy(out=site_i[:], in_=suf[:])

                gq = bigpool.tile([P, t, ROW], F16, tag="gq")
                for j in range(t):
                    g_.indirect_dma_start(
                        out=gq[:, j, :],
                        out_offset=None,
                        in_=table_d[:],
                        in_offset=bass.IndirectOffsetOnAxis(
                            ap=site_i[:, j : j + 1], axis=0
                        ),
                    )

                wuh = pool.tile([P, t], F16, tag="wuh")
                wvh = pool.tile([P, t], F16, tag="wvh")
                wuvh = pool.tile([P, t], F16, tag="wuvh")
                v.tensor_copy(out=wuh[:], in_=wu[:])
                v.tensor_copy(out=wvh[:], in_=wv[:])
                v.tensor_tensor(out=wuvh[:], in0=wuh[:], in1=wvh[:], op=MUL)

                A = gq[:, :, 0:6]
                Bd = gq[:, :, 6:12]
                Cd = gq[:, :, 12:18]
                Dd = gq[:, :, 18:24]
                wub = wuh[:, :, None].to_broadcast([P, t, 6])
                wvb = wvh[:, :, None].to_broadcast([P, t, 6])
                wuvb = wuvh[:, :, None].to_broadcast([P, t, 6])

                m1 = pool.tile([P, t, 6], F16, tag="m1")
                m2 = pool.tile([P, t, 6], F16, tag="m2")
                m3 = pool.tile([P, t, 6], F16, tag="m3")
                outt = pool.tile([P, t, 6], F32, tag="outt")
                v.tensor_tensor(out=m1[:], in0=Bd, in1=wub, op=MUL)
                v.tensor_tensor(out=m2[:], in0=Cd, in1=wvb, op=MUL)
                v.tensor_tensor(out=m3[:], in0=Dd, in1=wuvb, op=MUL)
                v.tensor_tensor(out=m1[:], in0=m1[:], in1=A, op=ADD)
                v.tensor_tensor(out=m2[:], in0=m2[:], in1=m3[:], op=ADD)
                v.tensor_tensor(out=outt[:], in0=m1[:], in1=m2[:], op=ADD)

                nc.sync.dma_start(
                    out=out_v[i], in_=outt[:].rearrange("p t c -> p (t c)")
                )

    nc.compile()
    return nc


_NC_CACHE = {}


def _get_nc():
    key = (PER_CORE, T_FREE)
    if key not in _NC_CACHE:
        _NC_CACHE[key] = build_nc(PER_CORE, T_FREE, bufs=3)
    return _NC_CACHE[key]


def kernel(inputs, tex, failv, _trace=False, _trace_kwargs=None):
    rays = np.ascontiguousarray(np.asarray(inputs, np.float32))
    assert rays.shape == (B_TOTAL, 3), rays.shape
    table = build_table(np.asarray(tex, np.float32), np.asarray(failv, np.float32))
    nc = _get_nc()
    in_maps = [
        {"rays": rays[i * PER_CORE : (i + 1) * PER_CORE], "table": table}
        for i in range(N_CORES)
    ]
    try:
        res = bass_utils.run_bass_kernel_spmd(
            nc, in_maps, list(range(N_CORES)), trace=_trace,
            **(_trace_kwargs or {}),
        )
    except Exception:
        if not _trace:
            raise
        res = bass_utils.run_bass_kernel_spmd(nc, in_maps, list(range(N_CORES)))
    out = np.concatenate([r["out"] for r in res.results], axis=0)
    if _trace:
        kernel.last_result = res
    return out

